# revision 47
# baseline (speedup 1.0000x reference)
"""KANvolution Trainium2 Bass kernel.

Math: the reference evaluates, per patch element x and per (f,c,ki,kj):
    K(x) = w_spline * sum_g basis_g(clip(x)) * cp_g  +  w_silu * silu(x)
with basis = normalized linear B-spline hats on a uniform 17-knot grid in
[-1,1].  The hat interpolant is piecewise-linear, so it is rewritten exactly as
    spline(x) = v0 + sum_{k=0..15} coef_k * relu(min(x,1) - g_k)
(v0 folds into the bias; the lower clip is redundant under the relu).  That
turns the whole module into a standard 3x3 valid conv over 17 feature maps
of x (16 relus + silu), contraction K = 9 taps x 544 (+1 bias row).

Sharding: 8 cores = (batch b, output-row half).  Each core gets a
(34, 66, 32) input slab (2-row halo) and produces (32, 64, 64).

Device pipeline per core (v4):
  - the host pre-computes every layout transform: x clipped+replicated to
    4 partition groups (relu features), tap-shifted raw-x sections for the
    silu k-tiles (9 taps packed into 3 k-tiles of K=96/97 incl. a bias
    row), and the [64f, spat] output is transposed back on host.
  - DMA dispatch is split across the two HW-DGE sequencers (SP: x/output,
    ACT: weights + silu sources) since each dma_start costs ~0.6us of
    serial sequencer time.
  - features (bf16): relu tiles t0-2 on DVE, t3 + the 3 silu packs on ACT,
    emitted in 4 column ranges so the matmul stream starts after ~1/4 of
    the feature work; ACT is ordered [relu r0, relu r1, silus...] so the
    late-arriving silu sources don't head-of-line-block the early relus.
  - matmuls (bf16): 39 k-tiles x 4 output chunks.  Chunks are processed in
    concurrent PAIRS on the two 64-column halves of the PE array (via out
    base partition -> tile_position col groups; both halves reference the
    SAME weight columns), so each ~213ns matmul slot retires two k-tiles.
  - tiny warmup matmuls + a dummy silu at t=0 pre-warm the HAM clock gate
    and the ACT function table while the input DMAs run.
"""

import numpy as np
from contextlib import ExitStack

import ml_dtypes
import concourse.bacc as bacc
import concourse.mybir as mybir
import concourse.tile as tile
from concourse.bass_utils import run_bass_kernel_spmd

# Problem constants (hardcoded per harness contract)
B, H, W, C, F = 4, 66, 66, 32, 64
KH = KW = 3
G = 16                      # spline intervals; G+1 = 17 knots
GRID_H = 2.0 / G            # 0.125
HO, WO = H - KH + 1, W - KW + 1          # 64, 64
N_CORES = 8
ROWS_PER_CORE = HO // 2                  # 32 output rows
IN_ROWS = ROWS_PER_CORE + KH - 1         # 34 input rows
SPAT = IN_ROWS * W                       # 2244 input spatial positions
SPAT_PAD = 2304                          # pack-tile column capacity
XOFF = 12                                # xr: gb (8) + bias (2) + pad (2)
XR_COLS = XOFF + SPAT
XR_SPLIT = XOFF + 1216                   # first xr DMA piece (ranges 0-1)
N_TAPS = KH * KW                         # 9
N_KTILES = 39                            # 36 relu + 3 silu packs
CHUNK_ROWS = 8                           # output rows per matmul chunk
N_CHUNKS = ROWS_PER_CORE // CHUNK_ROWS   # 4
NFREE = CHUNK_ROWS * WO                  # 512 moving-dim per matmul
RANGES = [0, 608, 1216, 1760, SPAT]      # feature column ranges
WARMUP_MMS = 140
W_PIECES = [0, 5, 12, 24, N_KTILES]      # w DMA pieces sized to match the
                                         # stream's k-tile consumption rate

_COMPILED = None


def _build_weights(control_points, w_spline, w_silu, bias):
    """Host-side transform of the KAN params into conv-as-matmul weights.

    Returns w_host [128, 39*64] bf16.  kt = tap*4+t (t-th relu g-group, row
    r*32+c encodes knot g=4t+r, channel c); kt 36+p = silu pack for taps
    {3p, 3p+1, 3p+2} (row 32r+c = tap 3p+r), with the bias row at row 96 of
    pack 0 (K=97; packs 1/2 use K=96).
    """
    cp = control_points.astype(np.float64)
    ws = w_spline.astype(np.float64)
    # hat interpolant values at the knots; the reference divides the hat
    # weights by (sum + 1e-8) with sum == 1, i.e. a uniform scale
    v = ws[..., None] * cp / (1.0 + 1e-8)          # (F, C, 3, 3, 17)
    s = np.diff(v, axis=-1) / GRID_H               # (F, C, 3, 3, 16) slopes
    coef = s.copy()
    coef[..., 1:] = s[..., 1:] - s[..., :-1]       # slope deltas at knots 1..15
    v0 = v[..., 0]                                 # value at x = -1
    bias_eff = bias.astype(np.float64) + v0.sum(axis=(1, 2, 3))   # (F,)
    wsilu = w_silu.astype(np.float64)

    w_all = np.zeros((N_KTILES, 128, F), dtype=np.float64)
    for i in range(KH):
        for j in range(KW):
            tap = i * KW + j
            for t in range(4):
                for r in range(4):
                    g = 4 * t + r
                    w_all[tap * 4 + t, r * 32:(r + 1) * 32, :] = coef[:, :, i, j, g].T
    for pk in range(2):                 # silu packs: taps 0-3 / 4-7
        for r in range(4):
            i, j = divmod(4 * pk + r, KW)
            w_all[36 + pk, r * 32:(r + 1) * 32, :] = wsilu[:, :, i, j].T
    i, j = divmod(8, KW)                # tap 8: K=32 (reads pack0 shifted)
    w_all[38, 0:32, :] = wsilu[:, :, i, j].T
    # duplicate as [Wkt | Wkt]: each PE column half needs its own LDWEIGHTS
    # source (walrus elides the second load if both halves share one AP)
    w_host = np.concatenate([w_all, w_all], axis=2)     # [39, 128, 128]
    w_host = w_host.transpose(1, 0, 2).reshape(128, N_KTILES * 2 * F)
    return np.ascontiguousarray(w_host.astype(ml_dtypes.bfloat16)), bias_eff


def _build_program():
    nc = bacc.Bacc("TRN2", target_bir_lowering=False, debug=False,
                   num_devices=N_CORES)
    f32 = mybir.dt.float32
    bf16 = mybir.dt.bfloat16

    # host-prepared inputs: xr = min(x,1) replicated to 4 partition groups
    # with the 4 per-partition knot biases appended as cols SPAT..SPAT+4;
    # xabc = raw x, section p = cols [p*SPAT_PAD, ...), rows 32r+c holding
    # x shifted by tap (i=p, j=r), row 96 zeros (becomes the const-1 row)
    xr_in = nc.declare_dram_parameter("xr", [128, XR_COLS], bf16, isOutput=False)
    # 128 partitions: DMAs with <128 partitions do not shard across the 16
    # DMA engines and crawl at single-queue bandwidth
    xabc_in = nc.declare_dram_parameter("xabc", [128, 2 * SPAT_PAD], bf16,
                                        isOutput=False)
    w_in = nc.declare_dram_parameter("w", [128, N_KTILES * 2 * F], bf16,
                                     isOutput=False)
    # output as [128, 2*512]: rows (q%2)*64+f, cols cp*512+s (host undoes)
    y_out = nc.declare_dram_parameter("y", [128, (N_CHUNKS // 2) * NFREE], f32,
                                      isOutput=True)

    with tile.TileContext(nc) as tc:
        with ExitStack() as ctx:
            sb = ctx.enter_context(tc.tile_pool(name="sb", bufs=1))
            ps = ctx.enter_context(tc.tile_pool(name="ps", bufs=2, space="PSUM"))
            ps_w = ctx.enter_context(tc.tile_pool(name="psw", bufs=1, space="PSUM"))
            ob = ctx.enter_context(tc.tile_pool(name="ob", bufs=2))

            # --- warmups first: ACT function table + PE HAM clock gate ---
            scratch = sb.tile([128, 8], bf16, tag="scratch")
            nc.vector.memset(scratch[:], 0.25)
            nc.scalar.activation(scratch[:, 4:8], scratch[:, 0:4],
                                 mybir.ActivationFunctionType.Silu)
            pw = ps_w.tile([4, 4], f32, tag="pw")
            for _ in range(WARMUP_MMS):
                nc.tensor.matmul(pw[:], scratch[:, 0:4], scratch[:, 0:4],
                                 start=True, stop=True)

            # --- input DMAs: xr on SP; w + xabc dispatched from ACT ---
            xr = sb.tile([128, XR_COLS], bf16, tag="xr")
            w_sb = sb.tile([128, N_KTILES * 2 * F], bf16, tag="w")
            xabc = sb.tile([128, 2 * SPAT_PAD], bf16, tag="xabc")

            def w_piece(piece):
                a = W_PIECES[piece] * 2 * F
                b = W_PIECES[piece + 1] * 2 * F
                nc.sync.dma_start(w_sb[:, a:b], w_in[:, a:b])

            # queue rings drain in dispatch order: the first matmul needs
            # xr piece 0 (features ranges 0-1) and w piece 0 only
            XR_R0 = XOFF + RANGES[1]
            nc.sync.dma_start(xr[:, :XR_R0], xr_in[:, :XR_R0])
            w_piece(0)
            nc.sync.dma_start(xr[:, XR_R0:XR_SPLIT], xr_in[:, XR_R0:XR_SPLIT])
            w_piece(1)
            nc.sync.dma_start(xr[:, XR_SPLIT:], xr_in[:, XR_SPLIT:])
            w_piece(2)
            w_piece(3)
            for p in range(2):
                o = p * SPAT_PAD
                nc.sync.dma_start(xabc[:, o:o + SPAT_PAD],
                                  xabc_in[:, o:o + SPAT_PAD])

            # knot biases + output bias ride along in xr as raw f32 bits
            gb = xr[:, 0:8].bitcast(f32)
            ob_bias = xr[:, 8:10].bitcast(f32)

            # --- features (bf16): relu t0-2 on DVE; t3 + silu packs on ACT.
            # ACT order: [relu-t3 r0, relu-t3 r1] before any silu so the
            # late xabc DMA doesn't head-of-line-block the early relus. ---
            feats = [sb.tile([128, SPAT], bf16, name=f"feat{t}",
                             tag=f"feat{t}") for t in range(4)]
            packs = sb.tile([128, 2 * SPAT_PAD], bf16, tag="packs")

            def relu_t3(r):
                cs = slice(RANGES[r], RANGES[r + 1])
                xs = slice(XOFF + RANGES[r], XOFF + RANGES[r + 1])
                nc.scalar.activation(feats[3][:, cs], xr[:, xs],
                                     mybir.ActivationFunctionType.Relu,
                                     bias=gb[:, 3:4], scale=1.0)

            def silus(r):
                cs0, cs1 = RANGES[r], RANGES[r + 1]
                for p in range(2):
                    o = p * SPAT_PAD
                    nc.scalar.activation(packs[:, o + cs0:o + cs1],
                                         xabc[:, o + cs0:o + cs1],
                                         mybir.ActivationFunctionType.Silu)

            relu_t3(0)
            relu_t3(1)
            silus(0)
            silus(1)
            relu_t3(2)
            relu_t3(3)
            silus(2)
            silus(3)

            for r in range(4):
                cs = slice(RANGES[r], RANGES[r + 1])
                xs = slice(XOFF + RANGES[r], XOFF + RANGES[r + 1])
                for t in range(3):      # DVE: (x + (-g)) max 0, per-part g
                    nc.vector.tensor_scalar(feats[t][:, cs], xr[:, xs],
                                            gb[:, t:t + 1], 0.0,
                                            mybir.AluOpType.add,
                                            mybir.AluOpType.max)

            # --- conv: 39 k-tiles, chunk pairs on PE column halves ---
            def rhs_for(kt, q):
                if kt < 36:
                    tap, t = divmod(kt, 4)
                    i, j = divmod(tap, KW)
                    base = (CHUNK_ROWS * q + i) * W
                    return (feats[t][:, base:base + CHUNK_ROWS * W]
                            .rearrange("p (r w) -> p r w", w=W)[:, :, j:j + WO])
                if kt < 38:             # silu packs 0/1: taps 0-3 / 4-7
                    p = kt - 36
                    base = p * SPAT_PAD + CHUNK_ROWS * q * W
                    return (packs[:, base:base + CHUNK_ROWS * W]
                            .rearrange("p (r w) -> p r w", w=W)[:, :, 0:WO])
                # tap 8 = pack0 rows 0-31 shifted by 2*W+2 columns
                base = CHUNK_ROWS * q * W + 2 * W + 2
                return (packs[0:32, base:base + CHUNK_ROWS * W]
                        .rearrange("p (r w) -> p r w", w=W)[:, :, 0:WO])

            for cp in range(N_CHUNKS // 2):
                po = ps.tile([128, NFREE], f32, tag="po")
                for kt in range(N_KTILES):
                    k = 32 if kt == 38 else 128
                    for half in range(2):
                        q = 2 * cp + half
                        nc.tensor.matmul(
                            po[F * half:F * (half + 1), :]
                            .rearrange("f (r w) -> f r w", w=WO),
                            w_sb[0:k, kt * 2 * F + F * half:
                                 kt * 2 * F + F * (half + 1)],
                            rhs_for(kt, q),
                            start=(kt == 0), stop=(kt == N_KTILES - 1),
                        )
                # two tiles so the ACT and DVE copies don't serialize on a
                # same-tile write-write dependency; the filter bias is
                # folded into these copies (per-partition bias operand)
                o_lo = ob.tile([128, NFREE], f32, tag="olo")
                o_hi = ob.tile([128, NFREE], f32, tag="ohi")
                nc.scalar.add(o_lo[0:F, :], po[0:F, :], ob_bias[0:F, :])
                nc.vector.tensor_scalar_add(o_hi[F:128, :], po[F:128, :],
                                            ob_bias[F:128, :])
                nc.sync.dma_start(
                    y_out[0:F, NFREE * cp:NFREE * (cp + 1)], o_lo[0:F, :])
                nc.sync.dma_start(
                    y_out[F:128, NFREE * cp:NFREE * (cp + 1)], o_hi[F:128, :])

    nc.compile()
    return nc


def _get_program():
    global _COMPILED
    if _COMPILED is None:
        _COMPILED = _build_program()
    return _COMPILED


def build_in_maps(x, control_points, w_spline, w_silu, bias):
    x = np.asarray(x, dtype=np.float32)
    w_host, bias_eff = _build_weights(
        np.asarray(control_points, dtype=np.float32),
        np.asarray(w_spline, dtype=np.float32),
        np.asarray(w_silu, dtype=np.float32),
        np.asarray(bias, dtype=np.float32))
    grid = np.linspace(-1.0, 1.0, G + 1, dtype=np.float64)

    bf = ml_dtypes.bfloat16
    in_maps = []
    for core in range(N_CORES):
        b, half = divmod(core, 2)
        r0 = half * ROWS_PER_CORE
        slab = np.zeros((C, SPAT_PAD), dtype=np.float32)
        slab[:, :SPAT] = x[b, r0:r0 + IN_ROWS].reshape(SPAT, C).T

        xr = np.zeros((128, XR_COLS), dtype=np.float32)
        xr[:, XOFF:XOFF + SPAT] = np.tile(np.minimum(slab[:, :SPAT], 1.0), (4, 1))
        xr = np.ascontiguousarray(xr.astype(bf))
        gbv = np.zeros((128, 4), dtype=np.float32)
        for t in range(4):
            for p in range(128):
                gbv[p, t] = -grid[4 * t + p // 32]
        # stash the f32 knot biases + filter bias bit-exactly in bf16 slots
        xr.view(np.uint16)[:, 0:8] = gbv.view(np.uint16)
        bias2 = np.tile(bias_eff.astype(np.float32), 2).reshape(128, 1)
        xr.view(np.uint16)[:, 8:10] = bias2.view(np.uint16)

        xabc = np.zeros((128, 2 * SPAT_PAD), dtype=np.float32)
        for p in range(2):
            for r in range(4):
                i, j = divmod(4 * p + r, KW)
                sh = i * W + j
                xabc[32 * r:32 * r + C,
                     p * SPAT_PAD:p * SPAT_PAD + SPAT_PAD - sh] = slab[:, sh:]
        in_maps.append({"xr": xr, "xabc": xabc.astype(bf), "w": w_host})
    return in_maps


def unshard_output(results):
    out = np.empty((B, HO, WO, F), dtype=np.float32)
    for core in range(N_CORES):
        b, half = divmod(core, 2)
        r0 = half * ROWS_PER_CORE
        y = results[core]["y"]                       # [128, 1024]
        # rows (q%2)*64+f, cols (q//2)*512 + r*64 + w
        y4 = y.reshape(2, F, 2, CHUNK_ROWS, WO)      # [q%2, f, cp, r, w]
        for q in range(N_CHUNKS):
            out[b, r0 + CHUNK_ROWS * q:r0 + CHUNK_ROWS * (q + 1)] = (
                y4[q % 2, :, q // 2].transpose(1, 2, 0))
    return out


def kernel(x, control_points, w_spline, w_silu, bias):
    in_maps = build_in_maps(x, control_points, w_spline, w_silu, bias)
    nc = _get_program()
    res = run_bass_kernel_spmd(nc, in_maps, list(range(N_CORES)))
    return unshard_output(res.results)
